# revision 13
# baseline (speedup 1.0000x reference)
"""Trainium2 Bass kernel for causal self-attention (B=2, T=2048, C=1024, H=16).

Sharding: tensor-parallel over heads x data-parallel over batch.
Each of the 8 cores handles one (batch b, head-group g) pair: b = core // 4,
g = core % 4, where a head group is 4 consecutive heads (heads 4g..4g+3).

Per-core pipeline (v2 — gap-free PE schedule):
  The kernel is one long interleaved instruction stream built around the
  attention dependency chain S -> exp(ACT) -> PV.  Per k-block step, BOTH
  heads of a pair share one [128,1024] S tile (hh0 cols 0-511, hh1 cols
  512-1023, different PSUM banks -> the two 64-row S matmuls run
  concurrently in different PE row groups), one exp covers both heads.
  All remaining PE work (QKV chains, v chains, output projection) lives in
  a deadline-ordered work-unit queue; units are pumped one matmul at a
  time into the gaps of the attention stream so the PE never idles waiting
  on ACT.  Startup is chunk-gated: x arrives in 8 column chunks and the
  first q/k/v chains consume each chunk as it lands.

  Normalize: 1/l on DVE straight out of PSUM row 64, partition-broadcast
  on GPSIMD, multiply on DVE.  Projection partials are summed on the host
  (the TP all-reduce) along with b_proj.
"""

import numpy as np
from collections import deque
from contextlib import ExitStack

import concourse.bass as bass
import concourse.tile as tile
from concourse import bacc, library_config, mybir
from concourse.bass import ts
from concourse.bass_utils import run_bass_kernel_spmd

F32 = mybir.dt.float32
BF16 = mybir.dt.bfloat16
AF = mybir.ActivationFunctionType
PSUM = bass.MemorySpace.PSUM


def _drain_copy(nc, pO, out_d, chunk, po, i):
    from concourse.bass import ts as _ts
    tb, cc = chunk
    ot = pO.tile([128, 512], F32, tag="ot", name="dot")
    if i % 2 == 0:
        nc.vector.tensor_copy(ot[:], po[:])
    else:
        nc.scalar.copy(ot[:], po[:])
    nc.sync.dma_start(out_d[_ts(tb, 128), _ts(cc, 512)], ot[:])

B, T, C, H = 2, 2048, 1024, 16
HD = C // H              # 64
HPC = 4                  # heads per core
PAIRS = 2                # head pairs per core
CI = C // 128            # 8 contraction chunks
TB = T // 128            # 16 t-blocks
NQC = T // 512           # 4 q-chunks
N_CORES = 8

IO_DT = BF16
QKV_DT = BF16
P_DT = BF16

FILL_PER_STEP = 4        # filler matmuls pumped per attention k-block step
FILL_PER_NORM = 10       # filler matmuls pumped around each normalize


def _emit(tc, nc, xT_d, wq_d, wk_d, wv_d, wp_d, out_d):
    ctx = ExitStack()
    with ctx:
        pers = ctx.enter_context(tc.tile_pool(name="pers", bufs=1))
        nc.gpsimd.load_library(library_config.attn)

        qT = [pers.tile([128, T], QKV_DT, name=f"qT{p}") for p in range(PAIRS)]
        kT = [pers.tile([128, T], QKV_DT, name=f"kT{p}") for p in range(PAIRS)]
        v_sb = [pers.tile([128, TB * 65], QKV_DT, name=f"v{h}") for h in range(HPC)]
        yT = [pers.tile([128, T], QKV_DT, name=f"yT{p}") for p in range(PAIRS)]
        wp_sb = pers.tile([128, 2048], IO_DT, name="wp")
        wq_sb = [pers.tile([128, 1024], IO_DT, name=f"wq{p}") for p in range(PAIRS)]
        wk_sb = [pers.tile([128, 1024], IO_DT, name=f"wk{p}") for p in range(PAIRS)]
        wv_sb = pers.tile([128, 2048], IO_DT, name="wv")
        mask_d = pers.tile([128, 128], P_DT, name="mask_d")
        xT_tiles = [pers.tile([128, T], IO_DT, name=f"xt{ci}") for ci in range(CI)]
        xT_sb = [t[:] for t in xT_tiles]

        # --- input DMA: weights for the first chains, then x chunks in order
        # (one ring -> chunks land roughly sequentially; chains consume
        # chunk ci as it arrives).  Late-needed weights go on the ACT ring.
        nc.sync.dma_start(wk_sb[0][:], wk_d[0])
        nc.sync.dma_start(wq_sb[0][:], wq_d[0])
        nc.sync.dma_start(wv_sb[:], wv_d[:])
        for ci in range(6):
            nc.sync.dma_start(xT_tiles[ci][:], xT_d[ts(ci, 128), :])
        nc.sync.dma_start(wk_sb[1][:], wk_d[1])
        nc.sync.dma_start(wq_sb[1][:], wq_d[1])
        for ci in range(6, CI):
            nc.sync.dma_start(xT_tiles[ci][:], xT_d[ts(ci, 128), :])
        nc.sync.dma_start(wp_sb[:], wp_d[:])

        # --- tiny setup: ACT exp table preload, causal mask, v ones column
        ones_f = pers.tile([128, 1], F32, name="ones_f")
        nc.gpsimd.memset(ones_f[:], 1.0)
        dummy = pers.tile([1, 1], F32, name="dummy")
        nc.scalar.activation(dummy[:], ones_f[0:1, 0:1], AF.Exp)
        for h in range(HPC):
            nc.vector.tensor_copy(
                v_sb[h][:].rearrange("p (t c) -> p t c", c=65)[:, :, 64:65],
                ones_f[:].unsqueeze(1).broadcast_to([128, TB, 1]),
            )
        mask_f = pers.tile([128, 128], F32, name="mask_f")
        nc.gpsimd.memset(mask_f[:], 1.0)
        nc.gpsimd.affine_select(
            out=mask_f[:], in_=mask_f[:],
            compare_op=mybir.AluOpType.is_ge, fill=0.0,
            base=0, channel_multiplier=-1, pattern=[[1, 128]],
        )
        nc.vector.tensor_copy(mask_d[:], mask_f[:])

        with (
            tc.tile_pool(name="psA", bufs=1, space=PSUM) as psA,
            tc.tile_pool(name="pP", bufs=3) as pP,
            tc.tile_pool(name="pN", bufs=2) as pN,
            tc.tile_pool(name="pO", bufs=3) as pO,
        ):
            # ---------- work units (generators; one emission per next()) ----
            def qk_unit(w_sb, dstT, qc):
                def gen():
                    ps = psA.tile([128, 512], F32, tag="fill", bufs=2,
                                  name="fqk")
                    for ci in range(CI):
                        nc.tensor.matmul(
                            ps[:], w_sb[:, ts(ci, 128)],
                            xT_sb[ci][:, ts(qc, 512)],
                            start=(ci == 0), stop=(ci == CI - 1),
                        )
                        yield
                    nc.vector.tensor_copy(dstT[:, ts(qc, 512)], ps[:])
                    yield
                return gen

            def v_unit(tb):
                def gen():
                    psv = psA.tile([128, 256], F32, tag="fill", bufs=2,
                                   name="fv")
                    for ci in range(CI):
                        nc.tensor.matmul(
                            psv[:], xT_sb[ci][:, ts(tb, 128)],
                            wv_sb[:, ts(ci, 256)],
                            start=(ci == 0), stop=(ci == CI - 1),
                        )
                        yield
                    for h in range(HPC):
                        nc.vector.tensor_copy(
                            v_sb[h][:, tb * 65: tb * 65 + 64],
                            psv[:, ts(h, 64)],
                        )
                    yield
                return gen

            def proj_unit(tb, cc):
                def gen():
                    po = psA.tile([128, 512], F32, tag="fill", bufs=2,
                                  name="fpo")
                    for p in range(PAIRS):
                        nc.tensor.matmul(
                            po[:], yT[p][:, ts(tb, 128)],
                            wp_sb[:, p * 1024 + cc * 512:
                                  p * 1024 + cc * 512 + 512],
                            start=(p == 0), stop=(p == PAIRS - 1),
                        )
                        yield
                    ot = pO.tile([128, 512], F32, tag="ot", name="ot")
                    nc.vector.tensor_copy(ot[:], po[:])
                    nc.sync.dma_start(out_d[ts(tb, 128), ts(cc, 512)], ot[:])
                    yield
                return gen

            # ---------- scheduler ----------
            unitq = deque()   # (label, gen_factory)
            done = set()
            cur = {"label": None, "gen": None}

            def _advance():
                # one emission; returns False when queue fully drained
                while True:
                    if cur["gen"] is None:
                        if not unitq:
                            return False
                        lbl, fac = unitq.popleft()
                        cur["label"], cur["gen"] = lbl, fac()
                    try:
                        next(cur["gen"])
                        return True
                    except StopIteration:
                        done.add(cur["label"])
                        cur["gen"] = None

            def pump(n):
                for _ in range(n):
                    if not _advance():
                        return

            def run_until_done(label):
                while label not in done:
                    if not _advance():
                        return

            def drain_all():
                while _advance():
                    pass

            # deadline-ordered queue (qc0 k/q/v handled in startup)
            for qc in range(NQC):
                if qc > 0:
                    unitq.append((f"k0c{qc}", qk_unit(wk_sb[0], kT[0], qc)))
                    unitq.append((f"q0c{qc}", qk_unit(wq_sb[0], qT[0], qc)))
                    for tb in range(4 * qc, 4 * qc + 4):
                        unitq.append((f"v{tb}", v_unit(tb)))
                unitq.append((f"k1c{qc}", qk_unit(wk_sb[1], kT[1], qc)))
                unitq.append((f"q1c{qc}", qk_unit(wq_sb[1], qT[1], qc)))
            for tb in range(4):
                done.add(f"v{tb}")

            # ---------- startup: chunk-gated qc0 chains (k0, q0, v0-3) ----
            with tc.tile_pool(name="psStart", bufs=1, space=PSUM) as psS0:
                psk = psS0.tile([128, 512], F32, tag="sk", name="psk")
                psq = psS0.tile([128, 512], F32, tag="sq", name="psq")
                psv4 = [psS0.tile([128, 256], F32, tag=f"sv{t}", name="psv")
                        for t in range(4)]
                for ci in range(CI):
                    st, sp = (ci == 0), (ci == CI - 1)
                    nc.tensor.matmul(psk[:], wk_sb[0][:, ts(ci, 128)],
                                     xT_sb[ci][:, 0:512], start=st, stop=sp)
                    nc.tensor.matmul(psq[:], wq_sb[0][:, ts(ci, 128)],
                                     xT_sb[ci][:, 0:512], start=st, stop=sp)
                    for t in range(4):
                        nc.tensor.matmul(psv4[t][:], xT_sb[ci][:, ts(t, 128)],
                                         wv_sb[:, ts(ci, 256)],
                                         start=st, stop=sp)
                nc.vector.tensor_copy(kT[0][:, 0:512], psk[:])
                nc.vector.tensor_copy(qT[0][:, 0:512], psq[:])
                for t in range(4):
                    for h in range(HPC):
                        nc.vector.tensor_copy(
                            v_sb[h][:, t * 65: t * 65 + 64],
                            psv4[t][:, ts(h, 64)],
                        )

            # ---------- main attention loop ----------
            with (
                tc.tile_pool(name="psS", bufs=1, space=PSUM) as psS,
                tc.tile_pool(name="psY", bufs=1, space=PSUM) as psY,
            ):
                def attn(p, qc, prefetch=None):
                    nkb = 4 * qc + 4
                    ypt = [psY.tile([65, 512], F32, tag=f"y{hh}", bufs=1,
                                    name=f"ypt{hh}") for hh in (0, 1)]
                    for kb in range(nkb):
                        if prefetch is not None and kb == max(0, nkb - 4):
                            run_until_done(prefetch)
                        col = max(0, (kb - 4 * qc) * 128)
                        sps = psS.tile([128, 1024], F32, tag="sp", bufs=2,
                                       name="sps")
                        for hh in (0, 1):
                            off = hh * 64
                            nc.tensor.matmul(
                                sps[:, hh * 512 + col: hh * 512 + 512],
                                kT[p][off:off + 64, ts(kb, 128)],
                                qT[p][off:off + 64,
                                      qc * 512 + col: (qc + 1) * 512],
                                start=True, stop=True,
                            )
                        pt = pP.tile([128, 1024], P_DT, tag="pt", name="pt")
                        # one exp covers both heads; skip cols left of the
                        # causal edge (hh0's dead region [0, col))
                        nc.scalar.activation(pt[:, col:1024],
                                             sps[:, col:1024], AF.Exp)
                        if kb >= 4 * qc:
                            for hh in (0, 1):
                                c0 = hh * 512 + col
                                nc.vector.tensor_mul(
                                    pt[:, c0:c0 + 128], pt[:, c0:c0 + 128],
                                    mask_d[:],
                                )
                        if kb >= 4 * qc and f"v{kb}" not in done:
                            run_until_done(f"v{kb}")
                        pump(FILL_PER_STEP)
                        for hh in (0, 1):
                            nc.tensor.matmul(
                                ypt[hh][0:65, col:512],
                                v_sb[2 * p + hh][:, kb * 65: (kb + 1) * 65],
                                pt[:, hh * 512 + col: hh * 512 + 512],
                                start=(kb == 0), stop=(kb == nkb - 1),
                            )
                    # normalize: yT = ypt_num * broadcast(1 / l)
                    for hh in (0, 1):
                        off = hh * 64
                        l_sb = pN.tile([1, 512], F32, tag="ls", name="ls")
                        nc.vector.tensor_copy(l_sb[:], ypt[hh][64:65, :])
                        rl = pN.tile([1, 512], F32, tag="rl", name="rl")
                        nc.vector.reciprocal_approx_fast(rl[:], l_sb[:])
                        lb = pN.tile([64, 512], F32, tag="lb", name="lb")
                        nc.gpsimd.partition_broadcast(lb[:], rl[:])
                        nc.vector.tensor_mul(
                            yT[p][off:off + 64, ts(qc, 512)],
                            ypt[hh][0:64, :], lb[:],
                        )

                for qc in range(NQC):
                    if qc > 0:
                        run_until_done(f"q0c{qc}")
                    attn(0, qc, prefetch=f"q1c{qc}")
                    pump(FILL_PER_NORM)
                    attn(1, qc,
                         prefetch=(f"q0c{qc + 1}" if qc < NQC - 1 else None))
                    pump(FILL_PER_NORM)
                    for tb in range(4 * qc + 3, 4 * qc - 1, -1):
                        if tb >= 12:
                            continue      # qc3 proj runs in the drain phase
                        for cc in (1, 0):
                            # ready now — run before later-deadline QKV units
                            # (barriers force those if the pump lags)
                            unitq.appendleft((f"pj{tb}_{cc}",
                                              proj_unit(tb, cc)))
                drain_all()

            # ---------- fast drain: last q-chunk's projection ----------
            # psS/psY are closed; use their banks for a wide proj pipeline
            # with copies alternating DVE/ACT (both idle post-attention).
            with tc.tile_pool(name="psD", bufs=1, space=PSUM) as psD:
                chunks = [(tb, cc) for tb in range(12, 16) for cc in range(2)]
                pos, ots = [], []
                for i, (tb, cc) in enumerate(chunks):
                    po = psD.tile([128, 512], F32, tag="po", bufs=6,
                                  name="dpo")
                    for p in range(PAIRS):
                        nc.tensor.matmul(
                            po[:], yT[p][:, ts(tb, 128)],
                            wp_sb[:, p * 1024 + cc * 512:
                                  p * 1024 + cc * 512 + 512],
                            start=(p == 0), stop=(p == PAIRS - 1),
                        )
                    pos.append(po)
                    if i >= 4:
                        _drain_copy(nc, pO, out_d, chunks[i - 4], pos[i - 4],
                                    i - 4)
                for i in range(len(chunks) - 4, len(chunks)):
                    _drain_copy(nc, pO, out_d, chunks[i], pos[i], i)


_NC_CACHE = None


def _build():
    global _NC_CACHE
    if _NC_CACHE is not None:
        return _NC_CACHE
    nc = bacc.Bacc("TRN2", target_bir_lowering=False, debug=False,
                   num_devices=N_CORES)
    xT_d = nc.dram_tensor("xT", [C, T], IO_DT, kind="ExternalInput")
    wq_d = nc.dram_tensor("wq", [PAIRS, 128, 1024], IO_DT, kind="ExternalInput")
    wk_d = nc.dram_tensor("wk", [PAIRS, 128, 1024], IO_DT, kind="ExternalInput")
    wv_d = nc.dram_tensor("wv", [128, 2048], IO_DT, kind="ExternalInput")
    wp_d = nc.dram_tensor("wp", [128, 2048], IO_DT, kind="ExternalInput")
    out_d = nc.dram_tensor("out", [T, C], F32, kind="ExternalOutput")

    with tile.TileContext(nc) as tc:
        _emit(tc, nc, xT_d, wq_d, wk_d, wv_d, wp_d, out_d)
    nc.compile()
    _NC_CACHE = nc
    return nc


def _pack_pair(m):
    # [1024, 128] -> lhsT chunks layout [128, 8*128]
    return np.ascontiguousarray(
        m.reshape(CI, 128, 128).transpose(1, 0, 2).reshape(128, 1024))


def _io_np(a):
    import ml_dtypes
    return np.ascontiguousarray(a.astype(ml_dtypes.bfloat16))


def _in_maps(x, w_attn, w_proj):
    x = np.asarray(x, dtype=np.float32)
    w_attn = np.asarray(w_attn, dtype=np.float32)
    w_proj = np.asarray(w_proj, dtype=np.float32)
    xT = [_io_np(x[b].T) for b in range(B)]
    maps = []
    for core in range(N_CORES):
        b, g = core // HPC, core % HPC
        cols = slice(g * 256, (g + 1) * 256)
        wk_full = w_attn[:, 0 * C:1 * C][:, cols]
        wq_full = w_attn[:, 1 * C:2 * C][:, cols] * np.float32(1.0 / np.sqrt(HD))
        wv_full = w_attn[:, 2 * C:3 * C][:, cols]
        wq_in = np.stack([_pack_pair(wq_full[:, p * 128:(p + 1) * 128])
                          for p in range(PAIRS)])
        wk_in = np.stack([_pack_pair(wk_full[:, p * 128:(p + 1) * 128])
                          for p in range(PAIRS)])
        wv_in = wv_full.reshape(CI, 128, 256).transpose(1, 0, 2).reshape(128, 2048)
        wp_in = (w_proj[g * 256:(g + 1) * 256, :]
                 .reshape(PAIRS, 128, 1024).transpose(1, 0, 2).reshape(128, 2048))
        maps.append({"xT": xT[b], "wq": _io_np(wq_in), "wk": _io_np(wk_in),
                     "wv": _io_np(wv_in), "wp": _io_np(wp_in)})
    return maps


def _assemble(results, b_proj):
    b_proj = np.asarray(b_proj, dtype=np.float32)
    out = np.zeros((B, T, C), dtype=np.float32)
    for core in range(N_CORES):
        out[core // HPC] += results[core]["out"]
    out += b_proj[None, None, :]
    return out


def kernel(x, w_attn, w_proj, b_proj):
    nc = _build()
    maps = _in_maps(x, w_attn, w_proj)
    res = run_bass_kernel_spmd(nc, maps, list(range(N_CORES)))
    return _assemble(res.results, b_proj)


def kernel_traced(x, w_attn, w_proj, b_proj):
    """Like kernel() but with NTFF tracing; returns (out, BassKernelResults)."""
    nc = _build()
    maps = _in_maps(x, w_attn, w_proj)
    res = run_bass_kernel_spmd(nc, maps, list(range(N_CORES)), trace=True)
    return _assemble(res.results, b_proj), res
